# revision 29
# baseline (speedup 1.0000x reference)
"""MIHash loss kernel for Trainium2 (8 NeuronCores, SPMD).

Math identical to the previous version: only R(8) = sum_j relu(w_ij - 8)
(w = 8 - pp/8, pp = phi@phi.T) needs a device pass; R(c<=7), R(9+) are
host-exact/zero.  relu(w-8) = relu(-pp)/8, so the device reduces
relu(-pp) row-wise (and class-band masked) over all N^2 pairs.

New architecture — circular-symmetric halving of the reduction work:
pp is symmetric, so for block-pairs at circular block-distance d in
[2,32] only the forward ordered pair is computed; the reverse-side row
sums are recovered as COLUMN sums of f = relu(-pp) via a ones-weights
matmul on the PE (which is 2.4x cheaper per element than ACT/DVE).
d in {-1,0,1} (the 3-block window around the diagonal, which also
contains every same-class band) is computed two-sided with full row
passes + masked band passes, exactly like before.

Per core (8 row-blocks of 128 rows: local blocks 0-3 and 32-35 of the
per-core-rotated column space; rotation by 512*core makes the SPMD
program core-independent):
  - window (3 blocks = 384 cols): pp -> full pass (ACT, accum) + band
    scalar_tensor_tensor with a host mask (DVE, accum)
  - arc (31/30 blocks forward): pp [128,512] tiles -> f = relu(-pp)
    bf16 passes alternating ACT/DVE with accum_out (row sums), then
    ones[128,32]-weights matmuls sum f over rows into per-512-column
    PSUM accumulator slots (4 banks x 4 partition-quadrants, start=False
    accumulation; pre-zeroed by zero-weight matmuls).
  - colsum slots are copied PSUM->SBUF by ACT/DVE and DMA'd out strided;
    the host adds them (un-rotated) to the transpose rows' R8.
Post-compile, redundant consecutive LDWEIGHTS (same weights AP +
tile_position) are deleted from the IR — the PE keeps the loaded
weights, saving ~11us of serial PE time; their sem waits move to the
following matmul.
Host does O(N*nbins) pre/post-processing (sort, second differences,
entropies) in float64.
"""

import os
import numpy as np
import ml_dtypes

import bass_rust
import concourse.bass as bass
import concourse.mybir as mybir
import concourse.tile as tile
from concourse import bacc
from concourse.bass_utils import run_bass_kernel_spmd

N = 8192
NBIT = 64
KPAD = 128
NCORES = 8
NBLK = 64                      # global 128-row blocks
LBLOCKS = [0, 1, 2, 3, 32, 33, 34, 35]   # local block indices per core
WIN = 384                      # window width (3 blocks)
MAXSEG = 129                   # window supports class sizes <= 129
NBINS = 16
EPS = 1e-7

F32 = mybir.dt.float32
F16 = mybir.dt.float16
BF16 = mybir.dt.bfloat16

# phiT DMA pieces (local col ranges), in issue order: first two are the
# block-0/1 window + arc head, so the PE can start ~1.7us in.
PIECES = [(7680, 8192), (0, 512), (512, 1536), (1536, 2560),
          (2560, 3584), (3584, 4608), (4608, 6656), (6656, 7680)]
PBOUNDS = sorted({b for p in PIECES for b in p})

_PROGRAM_CACHE = {}


def _arc_tiles(L):
    """Forward-arc col ranges for local block L, split at 512 boundaries."""
    units = 31 if L < 4 else 30
    a0, a1 = 128 * (L + 2), 128 * (L + 2) + 128 * units
    segs = [(a0, min(a1, N))]
    if a1 > N:
        segs.append((0, a1 - N))
    tiles = []
    for lo, hi in segs:
        x = lo
        while x < hi:
            nx = min(hi, (x // 512 + 1) * 512)
            tiles.append((x, nx))
            x = nx
    return tiles


def _split_at_pieces(lo, hi):
    """Split [lo,hi) at DMA-piece boundaries."""
    cuts = [lo] + [b for b in PBOUNDS if lo < b < hi] + [hi]
    return list(zip(cuts[:-1], cuts[1:]))


_ACT_512 = 797.0    # ACT per-512-col pass incl. ACTIVATION_READ_ACCUMULATOR
_DVE_512 = 605.0    # DVE per-512-col pass (min+accum)


def _group_table():
    """ACT/DVE assignment per 512-column group.  All arc tiles of a given
    column group use the same engine, so each colsum slot accumulates a
    single sign (ACT tiles hold relu(-pp) >= 0, DVE tiles hold min(pp,0)
    <= 0; the host negates DVE groups).  Greedy balance by total columns,
    seeded with the fixed window (ACT) and band (DVE) work."""
    cols = [0] * 16
    for L in LBLOCKS:
        for (lo, hi) in _arc_tiles(L):
            cols[lo // 512] += hi - lo
    a_load = 8 * 684.0
    v_load = 8 * 545.0
    table = [True] * 16
    for G in sorted(range(16), key=lambda g: -cols[g]):
        w = cols[G] / 512.0
        if a_load + _ACT_512 * w <= v_load + _DVE_512 * w:
            table[G] = True
            a_load += _ACT_512 * w
        else:
            table[G] = False
            v_load += _DVE_512 * w
    return table


_GROUP_IS_ACT = _group_table()


def _engine_is_act(bi, t):
    lo, _hi = _arc_tiles(LBLOCKS[bi])[t]
    return _GROUP_IS_ACT[lo // 512]


def _shrink_ldweights(nc):
    """For consecutive InstLdweights with identical weights/position:
    (a) shrink the reload to 1 column (~3ns instead of ~94ns of PE time;
    re-loads array col 0 with identical data), and (b) demote its sem
    waits onto a PE-sequencer NoOp placed after it, so the PE's reorder
    window can pull the (now wait-free) weight load ahead of in-flight
    matmuls.  First-use ldweights keep their waits (they may guard the
    weight data itself)."""
    shrunk = 0
    for f in nc.m.functions:
        for blk in f.blocks:
            insts = blk.instructions
            last_key = None
            i = 0
            while i < len(insts):
                inst = insts[i]
                if type(inst).__name__ != "InstLdweights":
                    i += 1
                    continue
                ap0 = inst.ins[0]
                key = (repr(ap0), repr(inst.perf_mode),
                       repr(inst.is_transpose), repr(inst.tile_position),
                       repr(inst.tile_size))
                if key == last_key:
                    ap = list(ap0.ap)
                    inst.ins = [mybir.PhysicalAccessPattern(
                        ap=[list(ap[0])] + [[1, 1]],
                        offset=ap0.offset, dtype=ap0.dtype,
                        memref=ap0.memref, memsetref=ap0.memsetref,
                    )]
                    if inst.tile_size is not None:
                        inst.tile_size = (inst.tile_size[0], 1)
                    si = inst.sync_info
                    if si is not None and si.on_wait:
                        nop = mybir.InstNoOp(
                            name=f"{inst.name}_w", ins=[], outs=[])
                        nop.engine = mybir.EngineType.PE
                        nop.sync_info = bass_rust.SyncInfo(
                            on_wait=list(si.on_wait), on_update=[])
                        inst.sync_info = bass_rust.SyncInfo(
                            on_wait=[], on_update=list(si.on_update))
                        insts.insert(i + 1, nop)
                        i += 1
                    shrunk += 1
                else:
                    last_key = key
                i += 1
    return shrunk


def _build_program():
    nc = bacc.Bacc(
        "TRN2", target_bir_lowering=False, debug=False, num_devices=NCORES
    )
    phiT_d = nc.dram_tensor("phiT", [KPAD, N], BF16, kind="ExternalInput")
    mask_d = nc.dram_tensor("mmask", [128, 8 * WIN], F16, kind="ExternalInput")
    rall_d = nc.dram_tensor("rall", [128, 128], F32, kind="ExternalOutput")
    rall2_d = nc.dram_tensor("rall2", [128, 128], F32, kind="ExternalOutput")
    csum_d = nc.dram_tensor("csum", [4, 2048], F32, kind="ExternalOutput")

    mn = mybir.AluOpType.min
    mx = mybir.AluOpType.max
    mult = mybir.AluOpType.mult
    relu = mybir.ActivationFunctionType.Relu

    with tile.TileContext(nc) as tc:
        with (
            tc.tile_pool(name="const", bufs=1) as constp,
            tc.tile_pool(name="ftile", bufs=26) as fpool,
            tc.tile_pool(name="scrw", bufs=2) as scrwp,
            tc.tile_pool(name="scrb", bufs=2) as scrbp,
            tc.tile_pool(name="pp", bufs=4, space=bass.MemorySpace.PSUM) as ppp,
            tc.tile_pool(name="cs", bufs=1, space=bass.MemorySpace.PSUM) as csp,
        ):
            # constants (gpsimd memsets: Pool engine is otherwise idle)
            zeros512 = constp.tile([128, 512], BF16)
            nc.gpsimd.memset(zeros512[:], 0.0)
            ones32 = constp.tile([128, 32], BF16)
            nc.gpsimd.memset(ones32[:], 1.0)
            bias0 = constp.tile([128, 1], F32)
            nc.vector.memset(bias0[:], 0.0)

            csum = csp.tile([128, 2048], F32)
            # zero the colsum banks with zero-weight matmuls
            for b in range(4):
                nc.tensor.matmul(
                    csum[:, 512 * b: 512 * (b + 1)],
                    zeros512[:, 0:128], zeros512[:, :],
                    start=True, stop=False, skip_group_check=True,
                )

            # phiT pieces (separate tiles so sems are per-piece)
            ptiles = {}
            for (lo, hi) in PIECES:
                t = constp.tile([KPAD, hi - lo], BF16, name=f"p{lo}")
                nc.sync.dma_start(t[:], phiT_d[:, lo:hi])
                ptiles[(lo, hi)] = t

            def pslice(lo, hi):
                """SBUF AP for local cols [lo,hi) (must lie in one piece)."""
                for (a, b), t in ptiles.items():
                    if a <= lo and hi <= b:
                        return t[:, lo - a: hi - a]
                raise AssertionError((lo, hi))

            masks = constp.tile([128, 8 * WIN], F16)
            nc.sync.dma_start(masks[:], mask_d[:, :])

            rall_a = constp.tile([128, 128], F32)   # ACT accums
            rall_v = constp.tile([128, 128], F32)   # DVE accums

            csum_sb = constp.tile([128, 2048], F32)

            def ones_run(f_list):
                """Column-sum matmuls for a block's f tiles (all hold
                relu(-pp) >= 0), grouped by PSUM partition-quadrant to
                minimize weight switches."""
                for (f, lo, hi) in sorted(
                        f_list, key=lambda x: (x[1] // 512) // 4):
                    G = lo // 512
                    bank, quad = G % 4, G // 4
                    c0 = 512 * bank + (lo - 512 * G)
                    nc.tensor.matmul(
                        csum[32 * quad: 32 * quad + 32, c0: c0 + (hi - lo)],
                        ones32[:, :], f[:, 0: hi - lo],
                        start=False, stop=False, skip_group_check=True,
                        tile_position=(0, 32 * quad),
                    )

            prev_f = None
            for bi, L in enumerate(LBLOCKS):
                own = pslice(128 * L, 128 * L + 128)
                base = bi * 16

                with tc.tile_wait_until(3 * bi, enable=False):
                    # window pp
                    wt = ppp.tile([128, 512], F32, tag="pp")
                    w0 = (128 * L - 128) % N
                    x = 0
                    for (lo, hi) in ([(w0, N), (0, WIN - (N - w0))]
                                     if w0 + WIN > N else [(w0, w0 + WIN)]):
                        for (a, b) in _split_at_pieces(lo, hi):
                            nc.tensor.matmul(
                                wt[:, x: x + b - a], own, pslice(a, b),
                                start=True, stop=True,
                            )
                            x += b - a
                    assert x == WIN

                    # window reductions: full (ACT) + band (DVE)
                    scrw = scrwp.tile([128, WIN], BF16, tag="scrw")
                    nc.scalar.activation(
                        scrw[:], wt[:, 0:WIN], relu, bias=bias0[:], scale=-1.0,
                        accum_out=rall_a[:, base: base + 1],
                    )
                    scrb = scrbp.tile([128, WIN], F32, tag="scrb")
                    nc.vector.scalar_tensor_tensor(
                        scrb[:], wt[:, 0:WIN], 0.0,
                        masks[:, bi * WIN: (bi + 1) * WIN],
                        mn, mult,
                        accum_out=rall_v[:, base + 1: base + 2],
                    )

                    # arc pp + f passes (f = relu(-pp) on both engines)
                    f_list = []
                    for t, (lo, hi) in enumerate(_arc_tiles(L)):
                        w = hi - lo
                        pp = ppp.tile([128, 512], F32, tag="pp")
                        for (a, b) in _split_at_pieces(lo, hi):
                            nc.tensor.matmul(
                                pp[:, a - lo: b - lo], own, pslice(a, b),
                                start=True, stop=True,
                            )
                        f = fpool.tile([128, 512], BF16, tag="f")
                        acol = base + 2 + t
                        if _engine_is_act(bi, t):
                            nc.scalar.activation(
                                f[:, 0:w], pp[:, 0:w], relu,
                                bias=bias0[:], scale=-1.0,
                                accum_out=rall_a[:, acol: acol + 1],
                            )
                        else:
                            nc.vector.tensor_scalar(
                                f[:, 0:w], pp[:, 0:w], 0.0, None,
                                mn, mybir.AluOpType.add,
                                accum_out=rall_v[:, acol: acol + 1],
                            )
                        f_list.append((f, lo, hi))

                # prev block's column sums, pinned after this block's pp
                # matmuls so the PE never alternates weights
                if prev_f is not None:
                    with tc.tile_wait_until(3 * bi + 1, enable=False):
                        ones_run(prev_f)
                prev_f = f_list

            with tc.tile_wait_until(3 * len(LBLOCKS), enable=False):
                ones_run(prev_f)

            # flush colsums: PSUM -> SBUF (per-bank, split ACT/DVE), then
            # one strided DMA of the quadrant rows
            for b in range(4):
                sl = slice(512 * b, 512 * (b + 1))
                if b % 2 == 0:
                    nc.scalar.activation(
                        csum_sb[:, sl], csum[:, sl],
                        mybir.ActivationFunctionType.Copy,
                        bias=0.0, scale=1.0,
                    )
                else:
                    nc.vector.tensor_scalar(
                        csum_sb[:, sl], csum[:, sl], 0.0, None,
                        mybir.AluOpType.add,
                    )
            nc.sync.dma_start(csum_d[:, :], csum_sb[0:128:32, :])
            nc.sync.dma_start(rall_d[:], rall_v[:])
            nc.sync.dma_start(rall2_d[:], rall_a[:])

    nc.compile()
    _shrink_ldweights(nc)
    return nc


def _numpy_reference(u, y):
    """Exact fallback for non-one-hot y / out-of-range inputs."""
    u = u.astype(np.float64)
    y = y.astype(np.float64)
    n, nbits = u.shape
    aff = ((y @ y.T) > 0).astype(np.float64)
    np.fill_diagonal(aff, 0.0)
    xp = aff
    xn = 1.0 - aff
    phi = 2.0 / (1.0 + np.exp(-u)) - 1.0
    dist = (nbits - phi @ phi.T) * 0.5
    prCp = xp.sum(1) / (n - 1)
    prCn = 1.0 - prCp
    delta = nbits // NBINS
    pDCp = np.zeros((n, NBINS))
    pDCn = np.zeros((n, NBINS))
    for b in range(NBINS):
        mid = b * delta
        ind = (dist > mid - delta) & (dist <= mid + delta)
        pulse = np.where(ind, 1.0 - np.abs(dist - mid) / delta, 0.0)
        pDCp[:, b] = (pulse * xp).sum(1)
        pDCn[:, b] = (pulse * xn).sum(1)
    return _finish_loss(pDCp, pDCn, prCp, prCn, n)


def _finish_loss(pDCp, pDCn, prCp, prCn, n):
    pD = (pDCp + pDCn) / (n - 1)
    sum_p = pDCp.sum(1)
    sum_n = pDCn.sum(1)
    safe_p = np.where(sum_p > 0, sum_p, 1.0)
    safe_n = np.where(sum_n > 0, sum_n, 1.0)
    pDCp = np.where((sum_p > 0)[:, None], pDCp / safe_p[:, None], pDCp)
    pDCn = np.where((sum_n > 0)[:, None], pDCn / safe_n[:, None], pDCn)

    def ent(p):
        return -(p * np.log(p + EPS)).sum(1)

    loss = (ent(pD) - (prCp * ent(pDCp) + prCn * ent(pDCn))).sum()
    return np.array(loss, dtype=np.float32)


def kernel(u, y):
    u = np.ascontiguousarray(np.asarray(u), dtype=np.float32)
    y = np.asarray(y)
    assert u.shape == (N, NBIT)

    pos = y > 0
    if not (pos.sum(axis=1) == 1).all() or (y < 0).any():
        return _numpy_reference(u, np.asarray(y, np.float32))
    labels = pos.argmax(axis=1)

    phi = np.tanh(u / 2.0)
    phib16 = phi.astype(ml_dtypes.bfloat16)
    phib = phib16.astype(np.float64)
    B = (phib * phib).sum(axis=1).max()
    if B >= 16.0:
        return _numpy_reference(u, np.asarray(y, np.float32))

    perm = np.argsort(labels, kind="stable")
    labels_s = labels[perm]
    counts = np.bincount(labels_s, minlength=labels_s.max() + 1)
    starts = np.concatenate([[0], np.cumsum(counts)])
    seg_s = starts[labels_s]
    seg_e = starts[labels_s + 1]
    if int(counts.max()) > MAXSEG:
        return _numpy_reference(u, np.asarray(y, np.float32))

    if "prog" not in _PROGRAM_CACHE:
        _PROGRAM_CACHE["prog"] = _build_program()
    nc = _PROGRAM_CACHE["prog"]

    phiT = np.zeros((KPAD, N), dtype=ml_dtypes.bfloat16)
    phiT[:NBIT] = phib16[perm].T
    phi64 = phib[perm]
    s_all = phi64.sum(axis=0)
    T_host = 8.0 * N - (phi64 @ s_all) / 8.0
    diag_w = 8.0 - (phi64 * phi64).sum(axis=1) / 8.0
    ncls = len(counts)
    cls_sums = np.zeros((ncls, NBIT))
    np.add.at(cls_sums, labels_s, phi64)
    Tp_host = (
        8.0 * ((seg_e - seg_s).astype(np.float64) - 1.0)
        - ((phi64 * (cls_sums[labels_s] - phi64)).sum(axis=1)) / 8.0
    )

    in_maps = []
    for core in range(NCORES):
        phiT_rot = np.roll(phiT, -512 * core, axis=1)
        mm = np.zeros((128, 8, WIN), dtype=np.float16)
        for bi, L in enumerate(LBLOCKS):
            B_g = 4 * core + L
            grow = 128 * B_g + np.arange(128)          # global rows
            gcol = (128 * B_g - 128) + np.arange(WIN)  # unwrapped global cols
            m = ((gcol[None, :] >= seg_s[grow][:, None])
                 & (gcol[None, :] < seg_e[grow][:, None]))
            m[np.arange(128), grow - (128 * B_g - 128)] = False
            mm[:, bi, :] = m
        in_maps.append({
            "phiT": phiT_rot,
            "mmask": mm.reshape(128, 8 * WIN).astype(np.float16),
        })

    res = run_bass_kernel_spmd(nc, in_maps, list(range(NCORES)))
    if os.environ.get("KERNEL_PROFILE", "0") == "1":
        try:
            tres = run_bass_kernel_spmd(nc, in_maps, list(range(NCORES)),
                                        trace=True)
            print(f"HW exec time: {tres.exec_time_ns} ns")
            if tres.instructions_and_trace is not None:
                print(f"trace path: {tres.instructions_and_trace[1]}")
        except Exception as e:
            print(f"profiling unavailable: {e}")

    # ---- host postprocessing (float64) ----
    R8x8 = np.zeros(N)      # sum relu(-pp) row sums (x8 the R8 scale)
    Rp8x8 = np.zeros(N)     # band: sum (pp min 0)*mask  (negative)
    for core in range(NCORES):
        out = res.results[core]
        ra = out["rall2"].astype(np.float64)   # ACT
        rv = out["rall"].astype(np.float64)    # DVE
        cs = out["csum"].astype(np.float64)    # [4, 2048]

        for bi, L in enumerate(LBLOCKS):
            B_g = 4 * core + L
            rows = slice(128 * B_g, 128 * B_g + 128)
            base = bi * 16
            acc = ra[:, base]                  # window full (ACT)
            band = rv[:, base + 1]             # band (DVE, negative)
            ntiles = len(_arc_tiles(L))
            for t in range(ntiles):
                col = base + 2 + t
                if _engine_is_act(bi, t):
                    acc = acc + ra[:, col]
                else:
                    acc = acc - rv[:, col]   # DVE tiles hold sum min(pp,0)
            R8x8[rows] += acc
            Rp8x8[rows] += band

        # colsums: quad q row holds groups G = 4q+b at cols [512b, 512b+512);
        # DVE-assigned groups accumulated min(pp,0) -> negate
        cl = np.zeros(N)
        for q in range(4):
            for b in range(4):
                G = 4 * q + b
                sgn = 1.0 if _GROUP_IS_ACT[G] else -1.0
                cl[512 * G: 512 * G + 512] = sgn * cs[q, 512 * b: 512 * b + 512]
        gcols = (np.arange(N) + 512 * core) % N
        np.add.at(R8x8, gcols, cl)

    R8 = R8x8 / 8.0
    Rp8 = -Rp8x8 / 8.0

    n_mask = (seg_e - seg_s - 1).astype(np.float64)
    T = T_host
    Tp = Tp_host
    R7 = T - 7.0 * N + np.maximum(7.0 - diag_w, 0.0)
    Rp7 = Tp - 7.0 * n_mask

    H_all = np.zeros((N, NBINS))
    H_all[:, 6] = 7.0 * N - T + R7
    H_all[:, 7] = T - 6.0 * N - 2.0 * R7 + R8
    H_all[:, 8] = R7 - 2.0 * R8
    H_all[:, 9] = R8

    H_p = np.zeros((N, NBINS))
    H_p[:, 6] = 7.0 * n_mask - Tp + Rp7
    H_p[:, 7] = Tp - 6.0 * n_mask - 2.0 * Rp7 + Rp8
    H_p[:, 8] = Rp7 - 2.0 * Rp8
    H_p[:, 9] = Rp8

    H_all = np.maximum(H_all, 0.0)
    H_p = np.maximum(H_p, 0.0)
    H_n = np.maximum(H_all - H_p, 0.0)

    prCp = n_mask / (N - 1)
    prCn = 1.0 - prCp
    return _finish_loss(H_p, H_n, prCp, prCn, N)


# revision 30
# speedup vs baseline: 1.1885x; 1.1885x over previous
"""MIHash loss kernel for Trainium2 (8 NeuronCores, SPMD).

Math identical to the previous version: only R(8) = sum_j relu(w_ij - 8)
(w = 8 - pp/8, pp = phi@phi.T) needs a device pass; R(c<=7), R(9+) are
host-exact/zero.  relu(w-8) = relu(-pp)/8, so the device reduces
relu(-pp) row-wise (and class-band masked) over all N^2 pairs.

New architecture — circular-symmetric halving of the reduction work:
pp is symmetric, so for block-pairs at circular block-distance d in
[2,32] only the forward ordered pair is computed; the reverse-side row
sums are recovered as COLUMN sums of f = relu(-pp) via a ones-weights
matmul on the PE (which is 2.4x cheaper per element than ACT/DVE).
d in {-1,0,1} (the 3-block window around the diagonal, which also
contains every same-class band) is computed two-sided with full row
passes + masked band passes, exactly like before.

Per core (8 row-blocks of 128 rows: local blocks 0-3 and 32-35 of the
per-core-rotated column space; rotation by 512*core makes the SPMD
program core-independent):
  - window (3 blocks = 384 cols): pp -> full pass (ACT, accum) + band
    scalar_tensor_tensor with a host mask (DVE, accum)
  - arc (31/30 blocks forward): pp [128,512] tiles -> f = relu(-pp)
    bf16 passes alternating ACT/DVE with accum_out (row sums), then
    ones[128,32]-weights matmuls sum f over rows into per-512-column
    PSUM accumulator slots (4 banks x 4 partition-quadrants, start=False
    accumulation; pre-zeroed by zero-weight matmuls).
  - colsum slots are copied PSUM->SBUF by ACT/DVE and DMA'd out strided;
    the host adds them (un-rotated) to the transpose rows' R8.
Post-compile, redundant consecutive LDWEIGHTS (same weights AP +
tile_position) are deleted from the IR — the PE keeps the loaded
weights, saving ~11us of serial PE time; their sem waits move to the
following matmul.
Host does O(N*nbins) pre/post-processing (sort, second differences,
entropies) in float64.
"""

import os
import numpy as np
import ml_dtypes

import bass_rust
import concourse.bass as bass
import concourse.mybir as mybir
import concourse.tile as tile
from concourse import bacc
from concourse.bass_utils import run_bass_kernel_spmd

N = 8192
NBIT = 64
KPAD = 128
NCORES = 8
NBLK = 64                      # global 128-row blocks
LBLOCKS = [0, 1, 2, 3, 32, 33, 34, 35]   # local block indices per core
WIN = 384                      # window width (3 blocks)
MAXSEG = 129                   # window supports class sizes <= 129
NBINS = 16
EPS = 1e-7

F32 = mybir.dt.float32
F16 = mybir.dt.float16
BF16 = mybir.dt.bfloat16

# phiT DMA pieces (local col ranges), in issue order: first two are the
# block-0/1 window + arc head, so the PE can start ~1.7us in.
PIECES = [(7680, 8192), (0, 512), (512, 1536), (1536, 2560),
          (2560, 3584), (3584, 4608), (4608, 6656), (6656, 7680)]
PBOUNDS = sorted({b for p in PIECES for b in p})

_PROGRAM_CACHE = {}


def _arc_tiles(L):
    """Forward-arc col ranges for local block L, split at 512 boundaries."""
    units = 31 if L < 4 else 30
    a0, a1 = 128 * (L + 2), 128 * (L + 2) + 128 * units
    segs = [(a0, min(a1, N))]
    if a1 > N:
        segs.append((0, a1 - N))
    tiles = []
    for lo, hi in segs:
        x = lo
        while x < hi:
            nx = min(hi, (x // 512 + 1) * 512)
            tiles.append((x, nx))
            x = nx
    return tiles


def _split_at_pieces(lo, hi):
    """Split [lo,hi) at DMA-piece boundaries."""
    cuts = [lo] + [b for b in PBOUNDS if lo < b < hi] + [hi]
    return list(zip(cuts[:-1], cuts[1:]))


_ACT_512 = 797.0    # ACT per-512-col pass incl. ACTIVATION_READ_ACCUMULATOR
_DVE_512 = 605.0    # DVE per-512-col pass (min+accum)


def _group_table():
    """ACT/DVE assignment per 512-column group.  All arc tiles of a given
    column group use the same engine, so each colsum slot accumulates a
    single sign (ACT tiles hold relu(-pp) >= 0, DVE tiles hold min(pp,0)
    <= 0; the host negates DVE groups).  Greedy balance by total columns,
    seeded with the fixed window (ACT) and band (DVE) work."""
    cols = [0] * 16
    for L in LBLOCKS:
        for (lo, hi) in _arc_tiles(L):
            cols[lo // 512] += hi - lo
    a_load = 8 * 684.0
    v_load = 8 * 545.0
    table = [True] * 16
    for G in sorted(range(16), key=lambda g: -cols[g]):
        w = cols[G] / 512.0
        if a_load + _ACT_512 * w <= v_load + _DVE_512 * w:
            table[G] = True
            a_load += _ACT_512 * w
        else:
            table[G] = False
            v_load += _DVE_512 * w
    return table


_GROUP_IS_ACT = _group_table()


def _engine_is_act(bi, t):
    lo, _hi = _arc_tiles(LBLOCKS[bi])[t]
    return _GROUP_IS_ACT[lo // 512]


def _shrink_ldweights(nc):
    """For consecutive InstLdweights with identical weights/position:
    (a) shrink the reload to 1 column (~3ns instead of ~94ns of PE time;
    re-loads array col 0 with identical data), and (b) demote its sem
    waits onto a PE-sequencer NoOp placed after it, so the PE's reorder
    window can pull the (now wait-free) weight load ahead of in-flight
    matmuls.  First-use ldweights keep their waits (they may guard the
    weight data itself)."""
    shrunk = 0
    for f in nc.m.functions:
        for blk in f.blocks:
            insts = blk.instructions
            last_key = None
            i = 0
            while i < len(insts):
                inst = insts[i]
                if type(inst).__name__ != "InstLdweights":
                    i += 1
                    continue
                ap0 = inst.ins[0]
                key = (repr(ap0), repr(inst.perf_mode),
                       repr(inst.is_transpose), repr(inst.tile_position),
                       repr(inst.tile_size))
                if key == last_key:
                    ap = list(ap0.ap)
                    inst.ins = [mybir.PhysicalAccessPattern(
                        ap=[list(ap[0])] + [[1, 1]],
                        offset=ap0.offset, dtype=ap0.dtype,
                        memref=ap0.memref, memsetref=ap0.memsetref,
                    )]
                    if inst.tile_size is not None:
                        inst.tile_size = (inst.tile_size[0], 1)
                    si = inst.sync_info
                    if si is not None and si.on_wait:
                        nop = mybir.InstNoOp(
                            name=f"{inst.name}_w", ins=[], outs=[])
                        nop.engine = mybir.EngineType.PE
                        nop.sync_info = bass_rust.SyncInfo(
                            on_wait=list(si.on_wait), on_update=[])
                        inst.sync_info = bass_rust.SyncInfo(
                            on_wait=[], on_update=list(si.on_update))
                        insts.insert(i + 1, nop)
                        i += 1
                    shrunk += 1
                else:
                    last_key = key
                i += 1
    return shrunk


def _build_program():
    nc = bacc.Bacc(
        "TRN2", target_bir_lowering=False, debug=False, num_devices=NCORES
    )
    phiT_d = nc.dram_tensor("phiT", [KPAD, N], BF16, kind="ExternalInput")
    mask_d = nc.dram_tensor("mmask", [128, 8 * WIN], F16, kind="ExternalInput")
    rall_d = nc.dram_tensor("rall", [128, 128], F32, kind="ExternalOutput")
    rall2_d = nc.dram_tensor("rall2", [128, 128], F32, kind="ExternalOutput")
    csum_d = nc.dram_tensor("csum", [4, 2048], F32, kind="ExternalOutput")

    mn = mybir.AluOpType.min
    mx = mybir.AluOpType.max
    mult = mybir.AluOpType.mult
    relu = mybir.ActivationFunctionType.Relu

    with tile.TileContext(nc) as tc:
        with (
            tc.tile_pool(name="const", bufs=1) as constp,
            tc.tile_pool(name="ftile", bufs=26) as fpool,
            tc.tile_pool(name="scrw", bufs=2) as scrwp,
            tc.tile_pool(name="scrb", bufs=2) as scrbp,
            tc.tile_pool(name="pp", bufs=4, space=bass.MemorySpace.PSUM) as ppp,
            tc.tile_pool(name="cs", bufs=1, space=bass.MemorySpace.PSUM) as csp,
        ):
            # constants (gpsimd memsets: Pool engine is otherwise idle)
            zeros512 = constp.tile([128, 512], BF16)
            nc.gpsimd.memset(zeros512[:], 0.0)
            ones32 = constp.tile([128, 32], BF16)
            nc.gpsimd.memset(ones32[:], 1.0)
            bias0 = constp.tile([128, 1], F32)
            nc.vector.memset(bias0[:], 0.0)

            csum = csp.tile([128, 2048], F32)
            # zero the colsum banks with zero-weight matmuls
            for b in range(4):
                nc.tensor.matmul(
                    csum[:, 512 * b: 512 * (b + 1)],
                    zeros512[:, 0:128], zeros512[:, :],
                    start=True, stop=False, skip_group_check=True,
                )

            # phiT pieces (separate tiles so sems are per-piece)
            ptiles = {}
            for (lo, hi) in PIECES:
                t = constp.tile([KPAD, hi - lo], BF16, name=f"p{lo}")
                nc.sync.dma_start(t[:], phiT_d[:, lo:hi])
                ptiles[(lo, hi)] = t

            def pslice(lo, hi):
                """SBUF AP for local cols [lo,hi) (must lie in one piece)."""
                for (a, b), t in ptiles.items():
                    if a <= lo and hi <= b:
                        return t[:, lo - a: hi - a]
                raise AssertionError((lo, hi))

            masks = constp.tile([128, 8 * WIN], F16)
            nc.sync.dma_start(masks[:], mask_d[:, :])

            rall_a = constp.tile([128, 128], F32)   # ACT accums
            rall_v = constp.tile([128, 128], F32)   # DVE accums

            csum_sb = constp.tile([128, 2048], F32)

            def ones_run(f_list):
                """Column-sum matmuls for a block's f tiles (all hold
                relu(-pp) >= 0), grouped by PSUM partition-quadrant to
                minimize weight switches."""
                for (f, lo, hi) in sorted(
                        f_list, key=lambda x: (x[1] // 512) // 4):
                    G = lo // 512
                    bank, quad = G % 4, G // 4
                    c0 = 512 * bank + (lo - 512 * G)
                    nc.tensor.matmul(
                        csum[32 * quad: 32 * quad + 32, c0: c0 + (hi - lo)],
                        ones32[:, :], f[:, 0: hi - lo],
                        start=False, stop=False, skip_group_check=True,
                        tile_position=(0, 32 * quad),
                    )

            prev_f = None
            for bi, L in enumerate(LBLOCKS):
                own = pslice(128 * L, 128 * L + 128)
                base = bi * 16

                with tc.tile_wait_until(3 * bi):
                    # window pp
                    wt = ppp.tile([128, 512], F32, tag="pp")
                    w0 = (128 * L - 128) % N
                    x = 0
                    for (lo, hi) in ([(w0, N), (0, WIN - (N - w0))]
                                     if w0 + WIN > N else [(w0, w0 + WIN)]):
                        for (a, b) in _split_at_pieces(lo, hi):
                            nc.tensor.matmul(
                                wt[:, x: x + b - a], own, pslice(a, b),
                                start=True, stop=True,
                            )
                            x += b - a
                    assert x == WIN

                    # window reductions: full (ACT) + band (DVE)
                    scrw = scrwp.tile([128, WIN], BF16, tag="scrw")
                    nc.scalar.activation(
                        scrw[:], wt[:, 0:WIN], relu, bias=bias0[:], scale=-1.0,
                        accum_out=rall_a[:, base: base + 1],
                    )
                    scrb = scrbp.tile([128, WIN], F32, tag="scrb")
                    nc.vector.scalar_tensor_tensor(
                        scrb[:], wt[:, 0:WIN], 0.0,
                        masks[:, bi * WIN: (bi + 1) * WIN],
                        mn, mult,
                        accum_out=rall_v[:, base + 1: base + 2],
                    )

                    # arc pp + f passes (f = relu(-pp) on both engines)
                    f_list = []
                    for t, (lo, hi) in enumerate(_arc_tiles(L)):
                        w = hi - lo
                        pp = ppp.tile([128, 512], F32, tag="pp")
                        for (a, b) in _split_at_pieces(lo, hi):
                            nc.tensor.matmul(
                                pp[:, a - lo: b - lo], own, pslice(a, b),
                                start=True, stop=True,
                            )
                        f = fpool.tile([128, 512], BF16, tag="f")
                        acol = base + 2 + t
                        if _engine_is_act(bi, t):
                            nc.scalar.activation(
                                f[:, 0:w], pp[:, 0:w], relu,
                                bias=bias0[:], scale=-1.0,
                                accum_out=rall_a[:, acol: acol + 1],
                            )
                        else:
                            nc.vector.tensor_scalar(
                                f[:, 0:w], pp[:, 0:w], 0.0, None,
                                mn, mybir.AluOpType.add,
                                accum_out=rall_v[:, acol: acol + 1],
                            )
                        f_list.append((f, lo, hi))

                # prev block's column sums, pinned after this block's pp
                # matmuls so the PE never alternates weights
                if prev_f is not None:
                    with tc.tile_wait_until(3 * bi + 1):
                        ones_run(prev_f)
                prev_f = f_list

            with tc.tile_wait_until(3 * len(LBLOCKS)):
                ones_run(prev_f)

            # flush colsums: PSUM -> SBUF (per-bank, split ACT/DVE), then
            # one strided DMA of the quadrant rows
            for b in range(4):
                sl = slice(512 * b, 512 * (b + 1))
                if b % 2 == 0:
                    nc.scalar.activation(
                        csum_sb[:, sl], csum[:, sl],
                        mybir.ActivationFunctionType.Copy,
                        bias=0.0, scale=1.0,
                    )
                else:
                    nc.vector.tensor_scalar(
                        csum_sb[:, sl], csum[:, sl], 0.0, None,
                        mybir.AluOpType.add,
                    )
            nc.sync.dma_start(csum_d[:, :], csum_sb[0:128:32, :])
            nc.sync.dma_start(rall_d[:], rall_v[:])
            nc.sync.dma_start(rall2_d[:], rall_a[:])

    nc.compile()
    _shrink_ldweights(nc)
    return nc


def _numpy_reference(u, y):
    """Exact fallback for non-one-hot y / out-of-range inputs."""
    u = u.astype(np.float64)
    y = y.astype(np.float64)
    n, nbits = u.shape
    aff = ((y @ y.T) > 0).astype(np.float64)
    np.fill_diagonal(aff, 0.0)
    xp = aff
    xn = 1.0 - aff
    phi = 2.0 / (1.0 + np.exp(-u)) - 1.0
    dist = (nbits - phi @ phi.T) * 0.5
    prCp = xp.sum(1) / (n - 1)
    prCn = 1.0 - prCp
    delta = nbits // NBINS
    pDCp = np.zeros((n, NBINS))
    pDCn = np.zeros((n, NBINS))
    for b in range(NBINS):
        mid = b * delta
        ind = (dist > mid - delta) & (dist <= mid + delta)
        pulse = np.where(ind, 1.0 - np.abs(dist - mid) / delta, 0.0)
        pDCp[:, b] = (pulse * xp).sum(1)
        pDCn[:, b] = (pulse * xn).sum(1)
    return _finish_loss(pDCp, pDCn, prCp, prCn, n)


def _finish_loss(pDCp, pDCn, prCp, prCn, n):
    pD = (pDCp + pDCn) / (n - 1)
    sum_p = pDCp.sum(1)
    sum_n = pDCn.sum(1)
    safe_p = np.where(sum_p > 0, sum_p, 1.0)
    safe_n = np.where(sum_n > 0, sum_n, 1.0)
    pDCp = np.where((sum_p > 0)[:, None], pDCp / safe_p[:, None], pDCp)
    pDCn = np.where((sum_n > 0)[:, None], pDCn / safe_n[:, None], pDCn)

    def ent(p):
        return -(p * np.log(p + EPS)).sum(1)

    loss = (ent(pD) - (prCp * ent(pDCp) + prCn * ent(pDCn))).sum()
    return np.array(loss, dtype=np.float32)


def kernel(u, y):
    u = np.ascontiguousarray(np.asarray(u), dtype=np.float32)
    y = np.asarray(y)
    assert u.shape == (N, NBIT)

    pos = y > 0
    if not (pos.sum(axis=1) == 1).all() or (y < 0).any():
        return _numpy_reference(u, np.asarray(y, np.float32))
    labels = pos.argmax(axis=1)

    phi = np.tanh(u / 2.0)
    phib16 = phi.astype(ml_dtypes.bfloat16)
    phib = phib16.astype(np.float64)
    B = (phib * phib).sum(axis=1).max()
    if B >= 16.0:
        return _numpy_reference(u, np.asarray(y, np.float32))

    perm = np.argsort(labels, kind="stable")
    labels_s = labels[perm]
    counts = np.bincount(labels_s, minlength=labels_s.max() + 1)
    starts = np.concatenate([[0], np.cumsum(counts)])
    seg_s = starts[labels_s]
    seg_e = starts[labels_s + 1]
    if int(counts.max()) > MAXSEG:
        return _numpy_reference(u, np.asarray(y, np.float32))

    if "prog" not in _PROGRAM_CACHE:
        _PROGRAM_CACHE["prog"] = _build_program()
    nc = _PROGRAM_CACHE["prog"]

    phiT = np.zeros((KPAD, N), dtype=ml_dtypes.bfloat16)
    phiT[:NBIT] = phib16[perm].T
    phi64 = phib[perm]
    s_all = phi64.sum(axis=0)
    T_host = 8.0 * N - (phi64 @ s_all) / 8.0
    diag_w = 8.0 - (phi64 * phi64).sum(axis=1) / 8.0
    ncls = len(counts)
    cls_sums = np.zeros((ncls, NBIT))
    np.add.at(cls_sums, labels_s, phi64)
    Tp_host = (
        8.0 * ((seg_e - seg_s).astype(np.float64) - 1.0)
        - ((phi64 * (cls_sums[labels_s] - phi64)).sum(axis=1)) / 8.0
    )

    in_maps = []
    for core in range(NCORES):
        phiT_rot = np.roll(phiT, -512 * core, axis=1)
        mm = np.zeros((128, 8, WIN), dtype=np.float16)
        for bi, L in enumerate(LBLOCKS):
            B_g = 4 * core + L
            grow = 128 * B_g + np.arange(128)          # global rows
            gcol = (128 * B_g - 128) + np.arange(WIN)  # unwrapped global cols
            m = ((gcol[None, :] >= seg_s[grow][:, None])
                 & (gcol[None, :] < seg_e[grow][:, None]))
            m[np.arange(128), grow - (128 * B_g - 128)] = False
            mm[:, bi, :] = m
        in_maps.append({
            "phiT": phiT_rot,
            "mmask": mm.reshape(128, 8 * WIN).astype(np.float16),
        })

    res = run_bass_kernel_spmd(nc, in_maps, list(range(NCORES)))
    if os.environ.get("KERNEL_PROFILE", "0") == "1":
        try:
            tres = run_bass_kernel_spmd(nc, in_maps, list(range(NCORES)),
                                        trace=True)
            print(f"HW exec time: {tres.exec_time_ns} ns")
            if tres.instructions_and_trace is not None:
                print(f"trace path: {tres.instructions_and_trace[1]}")
        except Exception as e:
            print(f"profiling unavailable: {e}")

    # ---- host postprocessing (float64) ----
    R8x8 = np.zeros(N)      # sum relu(-pp) row sums (x8 the R8 scale)
    Rp8x8 = np.zeros(N)     # band: sum (pp min 0)*mask  (negative)
    for core in range(NCORES):
        out = res.results[core]
        ra = out["rall2"].astype(np.float64)   # ACT
        rv = out["rall"].astype(np.float64)    # DVE
        cs = out["csum"].astype(np.float64)    # [4, 2048]

        for bi, L in enumerate(LBLOCKS):
            B_g = 4 * core + L
            rows = slice(128 * B_g, 128 * B_g + 128)
            base = bi * 16
            acc = ra[:, base]                  # window full (ACT)
            band = rv[:, base + 1]             # band (DVE, negative)
            ntiles = len(_arc_tiles(L))
            for t in range(ntiles):
                col = base + 2 + t
                if _engine_is_act(bi, t):
                    acc = acc + ra[:, col]
                else:
                    acc = acc - rv[:, col]   # DVE tiles hold sum min(pp,0)
            R8x8[rows] += acc
            Rp8x8[rows] += band

        # colsums: quad q row holds groups G = 4q+b at cols [512b, 512b+512);
        # DVE-assigned groups accumulated min(pp,0) -> negate
        cl = np.zeros(N)
        for q in range(4):
            for b in range(4):
                G = 4 * q + b
                sgn = 1.0 if _GROUP_IS_ACT[G] else -1.0
                cl[512 * G: 512 * G + 512] = sgn * cs[q, 512 * b: 512 * b + 512]
        gcols = (np.arange(N) + 512 * core) % N
        np.add.at(R8x8, gcols, cl)

    R8 = R8x8 / 8.0
    Rp8 = -Rp8x8 / 8.0

    n_mask = (seg_e - seg_s - 1).astype(np.float64)
    T = T_host
    Tp = Tp_host
    R7 = T - 7.0 * N + np.maximum(7.0 - diag_w, 0.0)
    Rp7 = Tp - 7.0 * n_mask

    H_all = np.zeros((N, NBINS))
    H_all[:, 6] = 7.0 * N - T + R7
    H_all[:, 7] = T - 6.0 * N - 2.0 * R7 + R8
    H_all[:, 8] = R7 - 2.0 * R8
    H_all[:, 9] = R8

    H_p = np.zeros((N, NBINS))
    H_p[:, 6] = 7.0 * n_mask - Tp + Rp7
    H_p[:, 7] = Tp - 6.0 * n_mask - 2.0 * Rp7 + Rp8
    H_p[:, 8] = Rp7 - 2.0 * Rp8
    H_p[:, 9] = Rp8

    H_all = np.maximum(H_all, 0.0)
    H_p = np.maximum(H_p, 0.0)
    H_n = np.maximum(H_all - H_p, 0.0)

    prCp = n_mask / (N - 1)
    prCn = 1.0 - prCp
    return _finish_loss(H_p, H_n, prCp, prCn, N)
